# revision 1
# baseline (speedup 1.0000x reference)
"""Trainium2 Bass kernel for nn_DistanceLoss (contrastive loss over cosine
similarity matrices).

Math restructure (vs the reference):
  loss = [ sum_i i*ld[i] - sum_{i>j} pos[i,j] ] / n_terms
where ld = logsumexp_k(neg[i,k]).  pos = (p1 @ p1.T)/T is symmetric with
diagonal 1/T, so the strict-lower-triangular sum collapses to
  ( ||sum_i p1_i||^2 / T - B/T ) / 2,
which needs only the column-sum s of normalized batch1 -- the whole [B,B]
pos matmul is eliminated.  Only neg = p1n @ p2n.T needs real compute.

Sharding: rows of batch1 are split 8 ways; batch2 is replicated into each
core's input map.  Each core emits ld for its 512-row strip plus its
partial s; the host does the final (tiny) reduction in float64.

Per-core pipeline (all heavy compute in bf16, fp32 PSUM accumulation):
  - cast-DMA inputs fp32->bf16 (SWDGE)
  - row sum-of-squares via DVE tensor_tensor_reduce (accum_out)
  - 1/sqrt(x) as Exp(-0.5 * Ln(x)) on ACT (same table set as the main
    Exp/Ln, so a single table load for the whole kernel)
  - normalize+transpose b2 fused: PE matmul of each [128,128] block against
    diag(10/||row||) built from an identity input tile
  - main matmul: neg_strip[i,k] accumulated over 4 c-chunks into PSUM
  - ACT Exp with accum_out -> per-row partial sums of exp (fused rowsum)
  - final Ln -> log-denominators; DMA out [2,512] per core
"""

import math
import os

import numpy as np
import ml_dtypes

B = 4096
C = 512
NCORES = 8
R = B // NCORES          # 512 rows per core strip
MB = R // 128            # 4 strip row-blocks
NBLK = B // 128          # 32 batch2 row-blocks
CC = C // 128            # 4 contraction chunks
NQ = 4                   # b2 DMA chunks (8 blocks each)
NTG = NBLK // 2          # 16 transpose groups (2 blocks each)
NMG = NBLK // 4          # 8 main matmul groups (512 k each)
TEMP = 0.1
N_TERMS = B * (B - 1) // 2

_CACHE = {}

# small scheduling/balance knobs, read by build_bass at trace time
CFG = {
    "evac_mode": "split",   # "split" (A->ACT, B->DVE) | "act" | "dve"
    "sumsq_mode": "dve",    # "dve" | "mixed" (odd blocks on ACT Square)
    "dumps_bufs": 3,
    "pt_bufs": 4,
    "fuse_exp": False,
}


def build_bass(reps=1, use_fp8=True, parts="full"):
    """Build the single-core SPMD Bass program (same NEFF on all 8 cores).

    reps > 1 repeats the whole per-core pipeline (same inputs, same output)
    inside one NEFF -- used for differential wall-clock timing, since the
    axon tunnel's ~5 ms dispatch cost swamps a single ~40 us kernel.

    parts: "full" | "nomain" (skip main matmul + exp) | "dma" (loads only)
    -- ablation variants for locating the bottleneck."""
    import concourse.bass as bass
    import concourse.bacc as bacc
    import concourse.tile as tile
    from concourse import mybir
    from contextlib import ExitStack

    fp32 = mybir.dt.float32
    bf16 = mybir.dt.bfloat16
    fp8 = mybir.dt.float8e4
    AF = mybir.ActivationFunctionType
    ALU = mybir.AluOpType
    AX = mybir.AxisListType

    nc = bacc.Bacc("TRN2", target_bir_lowering=False, debug=False,
                   num_devices=NCORES)

    b1s = nc.dram_tensor("b1s", [R, C], fp32, kind="ExternalInput")
    b2 = nc.dram_tensor("b2", [B, C], fp32, kind="ExternalInput")
    ident = nc.dram_tensor("ident", [128, 128], bf16, kind="ExternalInput")
    out = nc.dram_tensor("out", [2, 512], fp32, kind="ExternalOutput")

    with tile.TileContext(nc) as tc, ExitStack() as ctx:
        sb = ctx.enter_context(tc.tile_pool(name="sb", bufs=1))
        dumps = ctx.enter_context(
            tc.tile_pool(name="dumps", bufs=CFG["dumps_bufs"]))
        pt = ctx.enter_context(
            tc.tile_pool(name="pt", bufs=CFG["pt_bufs"], space="PSUM"))
        pneg = ctx.enter_context(tc.tile_pool(name="pneg", bufs=3, space="PSUM"))

        b1n = sb.tile([128, MB, C], bf16, name="b1n")
        b2n = sb.tile([128, NBLK, C], bf16, name="b2n")
        identb = sb.tile([128, 128], bf16, name="identb")
        mmdt = fp8 if use_fp8 else bf16
        b2sT = sb.tile([128, CC, B], mmdt, name="b2sT")
        p1T = sb.tile([128, CC, R], mmdt, name="p1T")
        diag1 = sb.tile([128, MB, 128], bf16, name="diag1")
        diag2 = sb.tile([128, NBLK, 128], bf16, name="diag2")
        ssq1 = sb.tile([128, MB], fp32, name="ssq1")
        ssq2 = sb.tile([128, NBLK], fp32, name="ssq2")
        ln1 = sb.tile([128, MB], fp32, name="ln1")
        ln2 = sb.tile([128, NBLK], fp32, name="ln2")
        invn1 = sb.tile([128, MB], fp32, name="invn1")
        invn1b = sb.tile([128, MB], bf16, name="invn1b")
        invn2s = sb.tile([128, NBLK], fp32, name="invn2s")
        denoms = sb.tile([128, MB * NMG], fp32, name="denoms")
        denom4 = sb.tile([128, MB], fp32, name="denom4")
        ld = sb.tile([128, MB], fp32, name="ld")
        s_f32 = sb.tile([128, CC], fp32, name="s_f32")
        probe_t = sb.tile([128, NQ + 2], fp32, name="probe_t")

        do_stats = parts in ("full", "nomain")
        do_main = parts == "full"

        def emit_body(last):
            # ---- loads: b1 path first so PE gets work early -------------------
            nc.sync.dma_start(identb[:, :], ident.ap())
            nc.gpsimd.dma_start(
                b1n[:, :, :], b1s.ap().rearrange("(m p) c -> p m c", p=128))
            for q in range(NQ):
                nc.gpsimd.dma_start(
                    b2n[:, q * 8:(q + 1) * 8, :],
                    b2.ap().rearrange("(blk p) c -> p blk c", p=128)[:, q * 8:(q + 1) * 8, :])

            if not do_stats:
                # consume the DMAs so reps serialize; nothing else
                for q in range(NQ):
                    nc.vector.tensor_copy(probe_t[:, q:q + 1],
                                          b2n[:, q * 8 + 7, 0:1])
                nc.vector.tensor_copy(probe_t[:, NQ:NQ + 1], b1n[:, MB - 1, 0:1])
                if last:
                    nc.sync.dma_start(
                        out.ap()[1, :].rearrange("(cc p) -> p cc", p=128),
                        probe_t[:, 0:CC])
                return

            # ---- batch1: norms, diag, transpose, column sums ------------------
            for m in range(MB):
                dmp = dumps.tile([128, C], bf16, name="dmp1", tag="dump1")
                nc.vector.scalar_tensor_tensor(
                    out=dmp[:, :], in0=b1n[:, m, :], scalar=1.0, in1=b1n[:, m, :],
                    op0=ALU.mult, op1=ALU.mult,
                    accum_out=ssq1[:, m:m + 1])
            # invn1 = 16/sqrt(ssq1): the 16x keeps fp8 p1T values in the
            # normal range; the main exp divides it back out via scale=1/16.
            nc.scalar.activation(ln1[:, :], ssq1[:, :], AF.Ln,
                                 scale=(1.0 / 256.0) if use_fp8 else 1.0)
            nc.scalar.activation(invn1[:, :], ln1[:, :], AF.Exp, scale=-0.5)
            nc.vector.tensor_scalar(
                invn1b[:, :], invn1[:, :],
                (1.0 / 16.0) if use_fp8 else 1.0, None, op0=ALU.mult)
            for m in range(MB):
                nc.vector.tensor_scalar_mul(
                    diag1[:, m, :], identb[:, :], invn1[:, m:m + 1])

            # p1T[c, i] = b1[i, c] / ||b1_i||  (transpose via matmul w/ diag rhs)
            for cc in range(CC):
                ptile = pt.tile([128, 2, 256], fp32, name="ptile", tag="pt")
                for m in range(MB):
                    nc.tensor.matmul(
                        ptile[:, m // 2, (m % 2) * 128:(m % 2 + 1) * 128],
                        lhsT=b1n[:, m, cc * 128:(cc + 1) * 128],
                        rhs=diag1[:, m, :],
                        start=True, stop=True)
                nc.vector.tensor_copy(
                    p1T[:, cc, :], ptile[:, :, :].rearrange("p a b -> p (a b)"))

            # s_partial[c] = sum_i p1n[i, c]  (ones-free: rhs = invnorm column)
            psum_s = pt.tile([128, CC], fp32, name="psum_s", tag="pt")
            for cc in range(CC):
                for m in range(MB):
                    nc.tensor.matmul(
                        psum_s[:, cc:cc + 1],
                        lhsT=b1n[:, m, cc * 128:(cc + 1) * 128],
                        rhs=invn1b[:, m:m + 1],
                        start=(m == 0), stop=(m == MB - 1))
            nc.vector.tensor_copy(s_f32[:, :], psum_s[:, :])

            # ---- batch2: per-DMA-chunk stats so the pipeline streams ----------
            probe = sb.tile([128, NQ], fp32, name="probe")
            for q in range(NQ):
                # tiny regular-instruction read of this DMA chunk: it absorbs
                # the DMA-sem wait so the STT sumsq ops below carry at most one
                # wait (the S2S2D2_STT encoding has a single sync-wait slot)
                nc.vector.tensor_copy(probe[:, q:q + 1], b2n[:, q * 8, 0:1])
                for j in range(8):
                    blk = q * 8 + j
                    if CFG["sumsq_mode"] == "mixed" and j % 2 == 1:
                        dmp = dumps.tile([128, C], bf16, name="dmp2a", tag="dump2a")
                        nc.scalar.activation(
                            dmp[:, :], b2n[:, blk, :], AF.Square,
                            accum_out=ssq2[:, blk:blk + 1])
                    else:
                        dmp = dumps.tile([128, C], bf16, name="dmp2", tag="dump2")
                        nc.vector.scalar_tensor_tensor(
                            out=dmp[:, :], in0=b2n[:, blk, :], scalar=1.0,
                            in1=b2n[:, blk, :],
                            op0=ALU.mult, op1=ALU.mult,
                            accum_out=ssq2[:, blk:blk + 1])
                # 10/sqrt(x) == exp(-0.5 * ln(0.01 * x))
                nc.scalar.activation(ln2[:, q * 8:(q + 1) * 8],
                                     ssq2[:, q * 8:(q + 1) * 8], AF.Ln, scale=0.01)
                nc.scalar.activation(invn2s[:, q * 8:(q + 1) * 8],
                                     ln2[:, q * 8:(q + 1) * 8], AF.Exp, scale=-0.5)
                for j in range(8):
                    blk = q * 8 + j
                    nc.vector.tensor_scalar_mul(
                        diag2[:, blk, :], identb[:, :], invn2s[:, blk:blk + 1])

            # ---- main pipeline ------------------------------------------------
            def emit_tgroup(tg):
                # transpose blocks 2tg, 2tg+1 into b2sT[:, :, tg*256:(tg+1)*256]
                ttA = pt.tile([128, 2, 256], fp32, name="ttA", tag="pt")
                ttB = pt.tile([128, 2, 256], fp32, name="ttB", tag="pt")
                tts = [ttA, ttB]
                for j in range(2):
                    blk = tg * 2 + j
                    for cc in range(CC):
                        nc.tensor.matmul(
                            tts[cc // 2][:, cc % 2, j * 128:(j + 1) * 128],
                            lhsT=b2n[:, blk, cc * 128:(cc + 1) * 128],
                            rhs=diag2[:, blk, :],
                            start=True, stop=True)
                ksl = slice(tg * 256, (tg + 1) * 256)
                mode = CFG["evac_mode"]
                ev_a = nc.scalar.copy if mode in ("split", "act") else \
                    nc.vector.tensor_copy
                ev_b = nc.vector.tensor_copy if mode in ("split", "dve") else \
                    nc.scalar.copy
                ev_a(b2sT[:, 0:2, ksl], ttA[:, :, :])
                ev_b(b2sT[:, 2:4, ksl], ttB[:, :, :])

            def emit_mgroup_fused(mgp):
                for m in range(MB):
                    ntile = pneg.tile([128, 2, 512], fp32, name="ntile", tag="pneg")
                    for half in range(2):
                        mg = 2 * mgp + half
                        for kg in range(2):
                            nc.tensor.matmul(
                                ntile[:, half, :],
                                lhsT=p1T[:, 2 * kg:2 * kg + 2, m * 128:(m + 1) * 128],
                                rhs=b2sT[:, 2 * kg:2 * kg + 2, mg * 512:(mg + 1) * 512],
                                start=(kg == 0), stop=(kg == 1),
                                perf_mode=mybir.MatmulPerfMode.DoubleRow)
                    dmp = dumps.tile([128, 1024], bf16, name="dmpe", tag="dumpe")
                    col = m * (NMG // 2) + mgp
                    nc.scalar.activation(
                        dmp[:, :], ntile[:, :, :].rearrange("p a b -> p (a b)"),
                        AF.Exp, scale=1.0 / 16.0,
                        accum_out=denoms[:, col:col + 1])

            def emit_mgroup(mg):
                for m in range(MB):
                    ntile = pneg.tile([128, 512], fp32, name="ntile", tag="pneg")
                    if use_fp8:
                        for kg in range(2):
                            nc.tensor.matmul(
                                ntile[:, :],
                                lhsT=p1T[:, 2 * kg:2 * kg + 2, m * 128:(m + 1) * 128],
                                rhs=b2sT[:, 2 * kg:2 * kg + 2, mg * 512:(mg + 1) * 512],
                                start=(kg == 0), stop=(kg == 1),
                                perf_mode=mybir.MatmulPerfMode.DoubleRow)
                    else:
                        for cc in range(CC):
                            nc.tensor.matmul(
                                ntile[:, :],
                                lhsT=p1T[:, cc, m * 128:(m + 1) * 128],
                                rhs=b2sT[:, cc, mg * 512:(mg + 1) * 512],
                                start=(cc == 0), stop=(cc == CC - 1))
                    dmp = dumps.tile([128, 512], bf16, name="dmpe", tag="dumpe")
                    col = m * NMG + mg
                    nc.scalar.activation(
                        dmp[:, :], ntile[:, :], AF.Exp,
                        scale=(1.0 / 16.0) if use_fp8 else 1.0,
                        accum_out=denoms[:, col:col + 1])

            if do_main and CFG["fuse_exp"]:
                for tg in range(4):
                    emit_tgroup(tg)
                for mgp in range(NMG // 2):
                    for tg in range(4 * mgp + 4, min(4 * mgp + 8, NTG)):
                        emit_tgroup(tg)
                    emit_mgroup_fused(mgp)
            elif do_main:
                emit_tgroup(0)
                emit_tgroup(1)
                for mg in range(NMG):
                    if 2 * mg + 2 < NTG:
                        emit_tgroup(2 * mg + 2)
                    if 2 * mg + 3 < NTG:
                        emit_tgroup(2 * mg + 3)
                    emit_mgroup(mg)
            else:
                for tg in range(NTG):
                    emit_tgroup(tg)
                # consume b2sT so the transposes+evacs aren't dangling
                nc.vector.tensor_copy(probe_t[:, NQ + 1:NQ + 2],
                                      b2sT[:, 0, B - 1:B])

            # ---- epilogue -----------------------------------------------------
            if not do_main:
                if last:
                    nc.sync.dma_start(
                        out.ap()[1, :].rearrange("(cc p) -> p cc", p=128),
                        s_f32[:, :])
                return
            nden = NMG // 2 if CFG["fuse_exp"] else NMG
            for m in range(MB):
                nc.vector.tensor_reduce(
                    denom4[:, m:m + 1],
                    denoms[:, m * nden:(m + 1) * nden],
                    axis=AX.X, op=ALU.add)
            nc.scalar.activation(ld[:, :], denom4[:, :], AF.Ln)
            if last:
                nc.sync.dma_start(
                    out.ap()[0, :].rearrange("(m p) -> p m", p=128), ld[:, :])
                nc.sync.dma_start(
                    out.ap()[1, :].rearrange("(cc p) -> p cc", p=128), s_f32[:, :])

        for _rep in range(reps):
            emit_body(last=(_rep == reps - 1))

    nc.compile()
    return nc


def _get_nc(reps=1, use_fp8=True, parts="full"):
    key = ("nc", reps, use_fp8, parts, tuple(sorted(CFG.items())))
    if key not in _CACHE:
        _CACHE[key] = build_bass(reps, use_fp8, parts)
    return _CACHE[key]


def make_in_maps(batch1, batch2):
    batch1 = np.ascontiguousarray(np.asarray(batch1, dtype=np.float32))
    batch2 = np.ascontiguousarray(np.asarray(batch2, dtype=np.float32))
    eye = np.eye(128, dtype=ml_dtypes.bfloat16)
    return [
        {"b1s": np.ascontiguousarray(batch1[c * R:(c + 1) * R]),
         "b2": batch2, "ident": eye}
        for c in range(NCORES)
    ]


def combine(results):
    """Host-side gather: results[c]["out"] is [2, 512] fp32 per core."""
    lds = np.concatenate([np.asarray(results[c]["out"][0], np.float64)
                          for c in range(NCORES)])
    s = np.sum([np.asarray(results[c]["out"][1], np.float64)
                for c in range(NCORES)], axis=0)
    term1 = np.dot(np.arange(B, dtype=np.float64), lds)
    tri = (np.dot(s, s) / TEMP - B / TEMP) / 2.0
    return np.asarray((term1 - tri) / N_TERMS, dtype=np.float32)


def run_hw(in_maps, trace=False, **kwargs):
    from concourse.bass_utils import run_bass_kernel_spmd
    return run_bass_kernel_spmd(_get_nc(), in_maps,
                                core_ids=list(range(NCORES)),
                                trace=trace, **kwargs)


def kernel(batch1, batch2):
    res = run_hw(make_in_maps(batch1, batch2))
    return combine(res.results)



# revision 8
# speedup vs baseline: 1.6870x; 1.6870x over previous
"""Trainium2 Bass kernel for nn_DistanceLoss (contrastive loss over cosine
similarity matrices).

Math restructure (vs the reference):
  loss = [ sum_i i*ld[i] - sum_{i>j} pos[i,j] ] / n_terms
where ld[i] = log sum_k exp(neg[i,k]).  pos = (p1 @ p1.T)/T is symmetric
with diagonal 1/T, so the strict-lower-triangular sum collapses to
  ( ||sum_i p1_i||^2 / T - B/T ) / 2,
needing only the column-sum s of normalized batch1 -- the [B,B] pos matmul
is eliminated.  Only neg = p1n @ p2n.T needs real compute.

Sharding (4x2): core (g, h) takes batch1 rows [g*1024,(g+1)*1024) and
batch2 rows [h*2048,(h+1)*2048): 6MB of input DMA per core instead of 9MB
for the 1D row split.  Each core emits partial denominators
  part[i] = sum_{k in its half} exp(neg[i,k])
for its 1024 rows plus the partial column-sum s of p1n; the host adds the
two k-halves, takes ln, and does the (tiny) final reduction in float64.

Per-core pipeline:
  - inputs land fp32->bf16 via SWDGE cast-DMA in a 4-rows-per-partition
    interleaved layout (8KB contiguous reads per descriptor; the implied
    row/k permutation is harmless -- exp row-sums are permutation
    invariant, and the host unpermutes the per-row outputs)
  - batch1 is never normalized on device: raw bf16 slices transpose via
    identity-rhs PE matmuls straight to fp8 p1T, and 1/||b1_i|| folds into
    the per-partition `scale` AP of the main Exp activation
  - batch2: DVE sum-of-squares -> ACT Ln/Exp -> 10/||row|| -> diag-scaled
    PE transpose (normalize+transpose in one matmul) -> fp8 b2sT
  - main matmul fp8 DoubleRow (4x bf16 throughput), PSUM fp32
  - ACT Exp with accum_out over [128,1024] PSUM tiles = fused row-sums
  - a single explicit LoadActFuncSet of the combined exp+ln+copy table
    (the automatic chooser otherwise thrashes 11 table loads)
"""

import numpy as np
import ml_dtypes

B = 4096
C = 512
NCORES = 8
G = 4                     # batch1 row groups
H = 2                     # batch2 row groups
R1 = B // G               # 1024 batch1 rows per core
R2 = B // H               # 2048 batch2 rows per core
F = 4                     # rows interleaved per partition line
M1 = R1 // 512            # 2 b1 512-row blocks
Q2 = R2 // 512            # 4 b2 512-row chunks
NS1 = R1 // 128           # 8 b1 slices
NS2 = R2 // 128           # 16 b2 slices
CC = C // 128             # 4 contraction chunks
TEMP = 0.1
N_TERMS = B * (B - 1) // 2

_CACHE = {}


def build_bass():
    import concourse.bass as bass
    import concourse.bacc as bacc
    import concourse.tile as tile
    from concourse import mybir
    from contextlib import ExitStack

    fp32 = mybir.dt.float32
    bf16 = mybir.dt.bfloat16
    fp8 = mybir.dt.float8e4
    AF = mybir.ActivationFunctionType
    ALU = mybir.AluOpType

    nc = bacc.Bacc("TRN2", target_bir_lowering=False, debug=False,
                   num_devices=NCORES)

    b1s = nc.dram_tensor("b1s", [R1, C], fp32, kind="ExternalInput")
    b2s = nc.dram_tensor("b2s", [R2, C], fp32, kind="ExternalInput")
    ident = nc.dram_tensor("ident", [128, 128], bf16, kind="ExternalInput")
    out = nc.dram_tensor("out", [128, 12], fp32, kind="ExternalOutput")

    with tile.TileContext(nc) as tc, ExitStack() as ctx:
        sb = ctx.enter_context(tc.tile_pool(name="sb", bufs=1))
        dumps = ctx.enter_context(tc.tile_pool(name="dumps", bufs=3))
        pt = ctx.enter_context(tc.tile_pool(name="pt", bufs=3, space="PSUM"))
        pneg = ctx.enter_context(tc.tile_pool(name="pneg", bufs=2, space="PSUM"))
        ps = ctx.enter_context(tc.tile_pool(name="ps", bufs=1, space="PSUM"))

        b1n = sb.tile([128, M1, F, C], bf16, name="b1n")
        b2n = sb.tile([128, Q2, F, C], bf16, name="b2n")
        identb = sb.tile([128, 128], bf16, name="identb")
        p1T = sb.tile([128, CC, R1], fp8, name="p1T")
        b2sT = sb.tile([128, CC, R2], fp8, name="b2sT")
        diag2 = sb.tile([128, NS2, 128], bf16, name="diag2")
        ssq1 = sb.tile([128, NS1], fp32, name="ssq1")
        ssq2 = sb.tile([128, NS2], fp32, name="ssq2")
        ln1 = sb.tile([128, NS1], fp32, name="ln1")
        ln2 = sb.tile([128, NS2], fp32, name="ln2")
        invn1 = sb.tile([128, NS1], fp32, name="invn1")
        invn1b = sb.tile([128, NS1], bf16, name="invn1b")
        invn2 = sb.tile([128, NS2], fp32, name="invn2")
        denoms = sb.tile([128, NS1, 2], fp32, name="denoms")
        outbuf = sb.tile([128, 12], fp32, name="outbuf")

        # ---- input DMA: everything issued up front on the SWDGE queue ----
        nc.sync.dma_start(identb[:, :], ident.ap())
        b1src = b1s.ap().rearrange("(m p f) c -> p m f c", p=128, f=F)
        for m in range(M1):
            nc.gpsimd.dma_start(b1n[:, m, :, :], b1src[:, m, :, :])
        b2src = b2s.ap().rearrange("(q p f) c -> p q f c", p=128, f=F)
        for q in range(Q2):
            nc.gpsimd.dma_start(b2n[:, q, :, :], b2src[:, q, :, :])

        # ---- batch1: raw transposes (identity rhs) + stats ---------------
        # (GPSIMD cannot touch PSUM, so evacuations go on ACT/DVE: the b1
        # ones on ACT, which is otherwise idle this early; b2's on DVE.)
        def b1_transpose(s1):
            m, f = s1 // F, s1 % F
            ptile = pt.tile([128, CC, 128], fp32, name="pt1", tag="pt")
            for cc in range(CC):
                nc.tensor.matmul(
                    ptile[:, cc, :],
                    lhsT=b1n[:, m, f, cc * 128:(cc + 1) * 128],
                    rhs=identb[:, :], start=True, stop=True)
            nc.scalar.copy(
                p1T[:, :, s1 * 128:(s1 + 1) * 128], ptile[:, :, :])

        def b1_stats():
            for s1 in range(NS1):
                m, f = s1 // F, s1 % F
                dmp = dumps.tile([128, C], bf16, name="d1", tag="d1")
                nc.vector.scalar_tensor_tensor(
                    out=dmp[:, :], in0=b1n[:, m, f, :], scalar=1.0,
                    in1=b1n[:, m, f, :], op0=ALU.mult, op1=ALU.mult,
                    accum_out=ssq1[:, s1:s1 + 1])
            # invn1 = 1/||row|| = exp(-0.5*ln(ssq))
            nc.scalar.activation(ln1[:, :], ssq1[:, :], AF.Ln)
            nc.scalar.activation(invn1[:, :], ln1[:, :], AF.Exp, scale=-0.5)
            nc.vector.tensor_scalar(invn1b[:, :], invn1[:, :], 1.0, None,
                                    op0=ALU.mult)

        def s_matmuls():
            # psum_s[c] = sum_i p1n[i, c], accumulated across slices
            psum_s = ps.tile([128, CC], fp32, name="psum_s", tag="ps")
            for s1 in range(NS1):
                m, f = s1 // F, s1 % F
                for cc in range(CC):
                    nc.tensor.matmul(
                        psum_s[:, cc:cc + 1],
                        lhsT=b1n[:, m, f, cc * 128:(cc + 1) * 128],
                        rhs=invn1b[:, s1:s1 + 1],
                        start=(s1 == 0), stop=(s1 == NS1 - 1))
            return psum_s

        # ---- batch2 per-chunk stats + diag + transpose -------------------
        def b2_stats(q):
            for f in range(F):
                s2 = q * F + f
                dmp = dumps.tile([128, C], bf16, name="d2", tag="d2")
                nc.vector.scalar_tensor_tensor(
                    out=dmp[:, :], in0=b2n[:, q, f, :], scalar=1.0,
                    in1=b2n[:, q, f, :], op0=ALU.mult, op1=ALU.mult,
                    accum_out=ssq2[:, s2:s2 + 1])
            sl = slice(q * F, (q + 1) * F)
            # 10/||row|| = exp(-0.5*ln(0.01*ssq))
            nc.scalar.activation(ln2[:, sl], ssq2[:, sl], AF.Ln, scale=0.01)
            nc.scalar.activation(invn2[:, sl], ln2[:, sl], AF.Exp, scale=-0.5)
            for f in range(F):
                s2 = q * F + f
                nc.vector.tensor_scalar_mul(
                    diag2[:, s2, :], identb[:, :], invn2[:, s2:s2 + 1])

        def b2_transpose(q):
            for f in range(F):
                s2 = q * F + f
                ptile = pt.tile([128, CC, 128], fp32, name="pt2", tag="pt")
                for cc in range(CC):
                    nc.tensor.matmul(
                        ptile[:, cc, :],
                        lhsT=b2n[:, q, f, cc * 128:(cc + 1) * 128],
                        rhs=diag2[:, s2, :], start=True, stop=True)
                nc.vector.tensor_copy(
                    b2sT[:, :, s2 * 128:(s2 + 1) * 128], ptile[:, :, :])

        # ---- main matmul + fused exp/rowsum ------------------------------
        def main_pass(P, ms):
            for m in ms:
                ntile = pneg.tile([128, 2, 512], fp32, name="ntile", tag="pn")
                for kg in range(2):
                    for mgx in range(2):
                        nc.tensor.matmul(
                            ntile[:, mgx, :],
                            lhsT=p1T[:, 2 * kg:2 * kg + 2, m * 128:(m + 1) * 128],
                            rhs=b2sT[:, 2 * kg:2 * kg + 2,
                                     (P * 2 + mgx) * 512:(P * 2 + mgx + 1) * 512],
                            start=(kg == 0), stop=(kg == 1),
                            perf_mode=mybir.MatmulPerfMode.DoubleRow)
                dmp = dumps.tile([128, 1024], bf16, name="de", tag="de")
                nc.scalar.activation(
                    dmp[:, :], ntile[:, :, :].rearrange("p a b -> p (a b)"),
                    AF.Exp, scale=invn1[:, m:m + 1],
                    accum_out=denoms[:, m, P:P + 1])

        # ---- emission order (per-engine program order = pipeline) --------
        for s1 in range(NS1):
            b1_transpose(s1)
        b1_stats()
        psum_s = s_matmuls()
        b2_stats(0)
        b2_transpose(0)
        b2_stats(1)
        b2_transpose(1)
        main_pass(0, range(0, 4))
        b2_stats(2)
        b2_transpose(2)
        main_pass(0, range(4, NS1))
        b2_stats(3)
        b2_transpose(3)
        main_pass(1, range(0, NS1))

        # ---- epilogue ----------------------------------------------------
        nc.vector.tensor_copy(outbuf[:, 8:12], psum_s[:, :])
        nc.vector.tensor_tensor(
            out=outbuf[:, 0:8], in0=denoms[:, :, 0], in1=denoms[:, :, 1],
            op=ALU.add)
        nc.sync.dma_start(out.ap(), outbuf[:, :])

    # Pin the combined exp+ln+copy activation table before compiling: the
    # automatic chooser alternates natural_log/exp_and_others and inserts a
    # 1.3us table load around every Ln<->Exp transition otherwise.
    try:
        from concourse.hw_specs import get_activation_tables
        tables = get_activation_tables(nc.m.arch)
        set_id = next(
            i for i, (_, fns) in enumerate(tables.items())
            if {AF.Exp, AF.Ln, AF.Copy} <= fns)
    except Exception:
        set_id = 6  # natural_log_exp_and_others in the shipped act_info.json
    inst = mybir.InstLoadActFuncSet(
        name=nc.get_next_instruction_name(), ins=[], outs=[],
        act_func_set_id=set_id)
    inst.engine = mybir.EngineType.Activation
    nc.register_instruction(inst)
    nc.main_func.blocks[0].instructions.insert(0, inst)

    nc.compile()
    return nc


def _get_nc():
    if "nc" not in _CACHE:
        _CACHE["nc"] = build_bass()
    return _CACHE["nc"]


def make_in_maps(batch1, batch2):
    batch1 = np.ascontiguousarray(np.asarray(batch1, dtype=np.float32))
    batch2 = np.ascontiguousarray(np.asarray(batch2, dtype=np.float32))
    eye = np.eye(128, dtype=ml_dtypes.bfloat16)
    maps = []
    for c in range(NCORES):
        g, h = c // H, c % H
        maps.append({
            "b1s": np.ascontiguousarray(batch1[g * R1:(g + 1) * R1]),
            "b2s": np.ascontiguousarray(batch2[h * R2:(h + 1) * R2]),
            "ident": eye,
        })
    return maps


def _row_perm():
    # out[p, s1] corresponds to local row (s1//F)*512 + 4*p + (s1%F)
    p = np.arange(128)
    s1 = np.arange(NS1)
    rows = (s1[None, :] // F) * 512 + 4 * p[:, None] + (s1[None, :] % F)
    return rows  # [128, NS1]


def combine(results):
    rows = _row_perm()
    D = np.zeros((H, B), dtype=np.float64)
    s = np.zeros(C, dtype=np.float64)
    for c in range(NCORES):
        g, h = c // H, c % H
        o = np.asarray(results[c]["out"], np.float64)  # [128, 12]
        idx = g * R1 + rows  # [128, NS1] global rows
        D[h, idx.ravel()] += o[:, 0:NS1].ravel()
        if h == 0:
            # s[cc*128 + p] = o[p, 8+cc]
            s += o[:, 8:12].T.ravel()
    ld = np.log(D[0] + D[1])
    term1 = np.dot(np.arange(B, dtype=np.float64), ld)
    tri = (np.dot(s, s) / TEMP - B / TEMP) / 2.0
    return np.asarray((term1 - tri) / N_TERMS, dtype=np.float32)


def run_hw(in_maps, trace=False, **kwargs):
    from concourse.bass_utils import run_bass_kernel_spmd
    return run_bass_kernel_spmd(_get_nc(), in_maps,
                                core_ids=list(range(NCORES)),
                                trace=trace, **kwargs)


def kernel(batch1, batch2):
    res = run_hw(make_in_maps(batch1, batch2))
    return combine(res.results)
